# revision 15
# baseline (speedup 1.0000x reference)
"""Differentiable episodic memory retrieval kernel for Trainium2 (8 NeuronCores).

Shards mamba_states over batch (1 batch element per core); memory matrix and
projection weights are replicated. All device tensors use a feature-major
("transposed") layout [d, tokens] so every matmul contracts over the SBUF
partition dimension.

Math (per core, X = states^T [d, tok]):
  Q^T = Wq^T X + bq                  (f32r matmuls)
  c_t = 1/||Q_t||                    (Square + ones-matmul partition reduction)
  S^T[n,t] = K^T(d,n) . Q^T(d,t)     (bf16; K unnormalized)
  E = exp(S * c_t * kscale_n)        (kscale_n = 1/(sqrt(d)*||K_n||), ACT scale)
  w = E / sum_n E                    (ones-matmul sums, K=1 matmul broadcast)
  R^T = V^T w  (V includes bv; softmax weights sum to 1 so bias passes through)
  G = Wg1^T X + W2V^T w + bg         (W2V^T = V Wg2 precomputed in preamble)
  O = R + sigmoid(G) * (X - R)
"""

import numpy as np

import concourse.bass as bass
import concourse.mybir as mybir
import concourse.tile as tile
from concourse import bacc
from concourse.bass_utils import run_bass_kernel_spmd

B, T, D = 8, 4096, 1024
NS = 512          # memory slots
TB = 512          # tokens per block
NBLK = T // TB    # 8
NDT = D // 128    # 8 tiles along d
NST = NS // 128   # 4 tiles along slots
P = 128
H = D // 2

F32 = mybir.dt.float32
F32R = mybir.dt.float32r
BF16 = mybir.dt.bfloat16

_CACHE = {}
LAST_RESULTS = None


def _f32(ap):
    return ap.bitcast(F32)


def _build():
    from contextlib import ExitStack

    nc = bacc.Bacc("TRN2", target_bir_lowering=False, debug=False)

    # f32r dram tensors: fed straight into f32r matmuls (same bits as f32)
    xt = nc.dram_tensor("xt", [D, T], F32R, kind="ExternalInput").ap()
    memt = nc.dram_tensor("memt", [D, NS], F32R, kind="ExternalInput").ap()
    wq = nc.dram_tensor("wq", [D, D], F32R, kind="ExternalInput").ap()
    wk = nc.dram_tensor("wk", [D, D], F32R, kind="ExternalInput").ap()
    wv = nc.dram_tensor("wv", [D, D], F32R, kind="ExternalInput").ap()
    wg = nc.dram_tensor("wg", [2 * D, D], F32R, kind="ExternalInput").ap()
    bq = nc.dram_tensor("bq", [D], F32, kind="ExternalInput").ap()
    bk = nc.dram_tensor("bk", [D], F32, kind="ExternalInput").ap()
    bv = nc.dram_tensor("bv", [D], F32, kind="ExternalInput").ap()
    bg = nc.dram_tensor("bg", [D], F32, kind="ExternalInput").ap()
    ot = nc.dram_tensor("ot", [D, T], F32, kind="ExternalOutput").ap()

    with tile.TileContext(nc) as tc, ExitStack() as ctx:
        _body(nc, tc, ctx, xt, memt, wq, wk, wv, wg, bq, bk, bv, bg, ot)

    nc.compile()
    return nc


def _body(nc, tc, ctx, xt, memt, wq, wk, wv, wg, bq, bk, bv, bg, ot):
    Act = mybir.ActivationFunctionType

    singles = ctx.enter_context(tc.tile_pool(name="singles", bufs=1))
    wpool = ctx.enter_context(tc.tile_pool(name="weights", bufs=1))

    # --- constants -----------------------------------------------------------
    ones_col = singles.tile([P, 1], BF16)          # lhsT for partition sums
    nc.vector.memset(ones_col, 1.0)
    ones_row = singles.tile([1, P], F32)           # lhsT for partition bcast
    nc.vector.memset(ones_row, 1.0)

    # per-partition bias tiles: [p, t] = b[t*128 + p]
    bq_sb = singles.tile([P, NDT], F32)
    nc.sync.dma_start(out=bq_sb, in_=bq.rearrange("(t p) -> p t", p=P))
    bk_sb = singles.tile([P, NDT], F32)
    nc.sync.dma_start(out=bk_sb, in_=bk.rearrange("(t p) -> p t", p=P))
    bg_sb = singles.tile([P, NDT], F32)
    nc.sync.dma_start(out=bg_sb, in_=bg.rearrange("(t p) -> p t", p=P))
    bv_sb = singles.tile([P, NDT], F32)
    nc.sync.dma_start(out=bv_sb, in_=bv.rearrange("(t p) -> p t", p=P))
    # bv broadcast across partitions: [128, D]
    bv_bc = singles.tile([P, D], F32)
    nc.sync.dma_start(
        out=bv_bc,
        in_=bass.AP(tensor=bv.tensor, offset=bv.offset, ap=[[0, P], [1, D]]),
    )

    # --- resident weights ----------------------------------------------------
    wq_sb = [wpool.tile([P, D], F32R, tag=f"wq{i}", name="wq_sb") for i in range(NDT)]
    for k in range(NDT):
        nc.sync.dma_start(out=wq_sb[k], in_=wq[k * P:(k + 1) * P, :])
    wg_sb = [wpool.tile([P, D], F32R, tag=f"wg{i}", name="wg_sb") for i in range(NDT)]
    for k in range(NDT):
        nc.sync.dma_start(out=wg_sb[k], in_=wg[k * P:(k + 1) * P, :])

    # static attention operands produced by the preamble
    ksb = [wpool.tile([P, NS], BF16, tag=f"ksb{i}", name="ksb") for i in range(NDT)]
    vsb = [wpool.tile([P, D], BF16, tag=f"vsb{i}", name="vsb") for i in range(NST)]
    w2vt = [wpool.tile([P, D], BF16, tag=f"w2vt{i}", name="w2vt") for i in range(NST)]
    kscale = [wpool.tile([P, 1], F32, tag=f"ksc{i}", name="kscale") for i in range(NST)]

    # =========================================================================
    # Preamble: K / V projections of the memory matrix, W2V^T = V @ Wg2
    # =========================================================================
    with tc.tile_pool(name="pre", bufs=1) as pre, \
         tc.tile_pool(name="pre_ps", bufs=2, space="PSUM") as pre_ps, \
         tc.tile_pool(name="pre_tmp", bufs=2) as pre_tmp:
        mem_sb = [pre.tile([P, NS], F32R, tag=f"mem{i}", name="mem_sb")
                  for i in range(NDT)]
        for k in range(NDT):
            nc.sync.dma_start(out=mem_sb[k], in_=memt[k * P:(k + 1) * P, :])
        wk_sb = [pre.tile([P, D], F32R, tag=f"wk{i}", name="wk_sb")
                 for i in range(NDT)]
        for k in range(NDT):
            nc.sync.dma_start(out=wk_sb[k], in_=wk[k * P:(k + 1) * P, :])
        # wv and wg2 reuse the wk slots (sequential phases)
        wv_sb = [pre.tile([P, D], F32R, tag=f"wk{i}", name="wv_sb")
                 for i in range(NDT)]
        wg2_sb = [pre.tile([P, D], F32R, tag=f"wk{i}", name="wg2_sb")
                  for i in range(NDT)]

        bk_bc = pre.tile([P, D], F32, tag="bkbc")
        nc.sync.dma_start(
            out=bk_bc,
            in_=bass.AP(tensor=bk.tensor, offset=bk.offset, ap=[[0, P], [1, D]]),
        )

        # K^T feature-major [d, slots], bias added, cast to bf16
        for m in range(NDT):
            ps = pre_ps.tile([P, NS], F32, tag="pps", name="ps")
            for k in range(NDT):
                nc.tensor.matmul(
                    ps, wk_sb[k][:, m * P:(m + 1) * P], mem_sb[k],
                    start=(k == 0), stop=(k == NDT - 1),
                )
            nc.scalar.activation(
                out=ksb[m], in_=ps, func=Act.Identity, bias=bk_sb[:, m:m + 1],
            )

        # K slot-major (transient) -> per-slot 1/(sqrt(d)*||K_n||) scales
        for s in range(NST):
            ss = None
            for h in range(2):
                ps = pre_ps.tile([P, H], F32, tag="pps", name="ps")
                for k in range(NDT):
                    nc.tensor.matmul(
                        ps,
                        mem_sb[k][:, s * P:(s + 1) * P],
                        wk_sb[k][:, h * H:(h + 1) * H],
                        start=(k == 0), stop=(k == NDT - 1),
                    )
                kr = pre_tmp.tile([P, H], F32, tag="krow")
                nc.vector.tensor_add(out=kr, in0=ps, in1=bk_bc[:, h * H:(h + 1) * H])
                sq = pre_tmp.tile([P, H], BF16, tag="ksq")
                half_ss = pre_tmp.tile([P, 1], F32, tag=f"kss{h}", name="half_ss")
                nc.scalar.activation(
                    out=sq, in_=kr, func=Act.Square, accum_out=half_ss,
                )
                if h == 0:
                    ss = half_ss
                else:
                    tot = pre_tmp.tile([P, 1], F32, tag="ksstot")
                    nc.vector.tensor_add(out=tot, in0=ss, in1=half_ss)
                    ss = tot
            # kscale = 1 / sqrt(D * ||K_n||^2)
            root = pre_tmp.tile([P, 1], F32, tag="kroot")
            nc.scalar.activation(out=root, in_=ss, func=Act.Sqrt, scale=float(D))
            nc.vector.reciprocal(out=kscale[s], in_=root)

        # V slot-major [slots, d], bias added directly (softmax weights sum to
        # one, so R = w @ (V0 + bv) = w @ V0 + bv matches the reference)
        for k in range(NDT):
            nc.sync.dma_start(out=wv_sb[k], in_=wv[k * P:(k + 1) * P, :])
        for s in range(NST):
            vtmp = pre_tmp.tile([P, D], F32, tag="vtmp")
            for h in range(2):
                ps = pre_ps.tile([P, H], F32, tag="pps", name="ps")
                for k in range(NDT):
                    nc.tensor.matmul(
                        ps,
                        mem_sb[k][:, s * P:(s + 1) * P],
                        wv_sb[k][:, h * H:(h + 1) * H],
                        start=(k == 0), stop=(k == NDT - 1),
                    )
                nc.vector.tensor_add(
                    out=vtmp[:, h * H:(h + 1) * H], in0=ps,
                    in1=bv_bc[:, h * H:(h + 1) * H],
                )
            nc.vector.tensor_copy(out=vsb[s], in_=vtmp)

        # V^T feature-major (bf16, transient, bias included) for W2V^T
        vt_bf = [pre_tmp.tile([P, NS], BF16, tag=f"vt{i}", bufs=1, name="vt_bf")
                 for i in range(NDT)]
        for m in range(NDT):
            ps = pre_ps.tile([P, NS], F32, tag="pps", name="ps")
            for k in range(NDT):
                nc.tensor.matmul(
                    ps, wv_sb[k][:, m * P:(m + 1) * P], mem_sb[k],
                    start=(k == 0), stop=(k == NDT - 1),
                )
            nc.scalar.activation(
                out=vt_bf[m], in_=ps, func=Act.Identity, bias=bv_sb[:, m:m + 1],
            )

        # wg2 in bf16 (transient): ACT copy from f32r load
        for k in range(NDT):
            nc.sync.dma_start(out=wg2_sb[k], in_=wg[D + k * P:D + (k + 1) * P, :])
        wg2_bf = [pre_tmp.tile([P, D], BF16, tag=f"wg2b{i}", bufs=1, name="wg2_bf")
                  for i in range(NDT)]
        for k in range(NDT):
            nc.scalar.activation(out=wg2_bf[k], in_=_f32(wg2_sb[k]), func=Act.Copy)

        # W2V^T slot-major [slots, dout] = V @ Wg2   (bf16)
        for s in range(NST):
            for h in range(2):
                ps = pre_ps.tile([P, H], F32, tag="pps", name="ps")
                for k in range(NDT):
                    nc.tensor.matmul(
                        ps, vt_bf[k][:, s * P:(s + 1) * P],
                        wg2_bf[k][:, h * H:(h + 1) * H],
                        start=(k == 0), stop=(k == NDT - 1),
                    )
                nc.scalar.activation(
                    out=w2vt[s][:, h * H:(h + 1) * H], in_=ps, func=Act.Copy,
                )

    # =========================================================================
    # Main loop over token blocks
    # =========================================================================
    xpool = ctx.enter_context(tc.tile_pool(name="xt", bufs=10))
    qpool = ctx.enter_context(tc.tile_pool(name="q", bufs=3))
    qqpool = ctx.enter_context(tc.tile_pool(name="qq", bufs=3))
    epool = ctx.enter_context(tc.tile_pool(name="e", bufs=5))
    rpool = ctx.enter_context(tc.tile_pool(name="r", bufs=9))
    gpool = ctx.enter_context(tc.tile_pool(name="g", bufs=3))
    tpool = ctx.enter_context(tc.tile_pool(name="tmp", bufs=3))
    opool = ctx.enter_context(tc.tile_pool(name="o", bufs=4))
    bpool = ctx.enter_context(tc.tile_pool(name="bcast", bufs=1))
    spool = ctx.enter_context(tc.tile_pool(name="small", bufs=2))

    ps_acc = ctx.enter_context(tc.tile_pool(name="ps_acc", bufs=2, space="PSUM"))
    ps_s = ctx.enter_context(tc.tile_pool(name="ps_s", bufs=4, space="PSUM"))
    ps_sm = ctx.enter_context(tc.tile_pool(name="ps_sm", bufs=2, space="PSUM"))

    for blk in range(NBLK):
        col = slice(blk * TB, (blk + 1) * TB)

        xts = [xpool.tile([P, TB], F32R, tag="xt", name="xts") for _ in range(NDT)]
        for k in range(NDT):
            nc.sync.dma_start(out=xts[k], in_=xt[k * P:(k + 1) * P, col])

        # ---- Q projection + sum of squares ---------------------------------
        qss_ps = ps_sm.tile([1, TB], F32, tag="sm", name="qss_ps")
        qsb = []
        for m in range(NDT):
            ps = ps_acc.tile([P, TB], F32, tag="acc", name="ps")
            for k in range(NDT):
                nc.tensor.matmul(
                    ps, wq_sb[k][:, m * P:(m + 1) * P], xts[k],
                    start=(k == 0), stop=(k == NDT - 1),
                )
            q_m = qpool.tile([P, TB], BF16, tag="qsb", name="q_m")
            nc.scalar.activation(
                out=q_m, in_=ps, func=Act.Identity, bias=bq_sb[:, m:m + 1],
            )
            qq_m = qqpool.tile([P, TB], BF16, tag="qsq", name="qq_m")
            nc.scalar.activation(
                out=qq_m, in_=ps, func=Act.Square, bias=bq_sb[:, m:m + 1],
            )
            nc.tensor.matmul(
                qss_ps, ones_col, qq_m,
                start=(m == 0), stop=(m == NDT - 1),
            )
            qsb.append(q_m)

        # ---- per-token scale c = 1/||Q_t||, broadcast to 128 partitions ----
        qroot = spool.tile([1, TB], F32, tag="qroot")
        nc.scalar.activation(out=qroot, in_=qss_ps, func=Act.Sqrt)
        c_sb = spool.tile([1, TB], F32, tag="c")
        nc.vector.reciprocal(out=c_sb, in_=qroot)
        cb_ps = ps_sm.tile([P, TB], F32, tag="sm", name="cb_ps")
        nc.tensor.matmul(cb_ps, ones_row, c_sb, start=True, stop=True)
        cb_sb = bpool.tile([P, TB], F32, tag="cb_sb")
        nc.scalar.activation(out=cb_sb, in_=cb_ps, func=Act.Copy)

        # ---- scores^T [slots, tok] -> exp ----------------------------------
        s_ps = [ps_s.tile([P, TB], F32, tag="pss", name="s_ps") for _ in range(NST)]
        for m in range(NDT):
            for s in range(NST):
                nc.tensor.matmul(
                    s_ps[s], ksb[m][:, s * P:(s + 1) * P], qsb[m],
                    start=(m == 0), stop=(m == NDT - 1),
                )
        ss_ps = ps_sm.tile([1, TB], F32, tag="sm", name="ss_ps")
        esb = []
        for s in range(NST):
            e_in = tpool.tile([P, TB], F32, tag="tt", name="e_in")
            nc.vector.tensor_mul(out=e_in, in0=s_ps[s], in1=cb_sb)
            e_s = epool.tile([P, TB], BF16, tag="esb", name="e_s")
            nc.scalar.activation(out=e_s, in_=e_in, func=Act.Exp, scale=kscale[s])
            nc.tensor.matmul(
                ss_ps, ones_col, e_s, start=(s == 0), stop=(s == NST - 1),
            )
            esb.append(e_s)

        # ---- normalized weights w = E / sum --------------------------------
        rs_sb = spool.tile([1, TB], F32, tag="rs")
        nc.vector.reciprocal(out=rs_sb, in_=ss_ps)
        rb_ps = ps_sm.tile([P, TB], F32, tag="sm", name="rb_ps")
        nc.tensor.matmul(rb_ps, ones_row, rs_sb, start=True, stop=True)
        rb_sb = bpool.tile([P, TB], F32, tag="rb_sb")
        nc.scalar.activation(out=rb_sb, in_=rb_ps, func=Act.Copy)
        wsb = []
        for s in range(NST):
            w_s = epool.tile([P, TB], BF16, tag="wsb", name="w_s")
            nc.vector.tensor_mul(out=w_s, in0=esb[s], in1=rb_sb)
            wsb.append(w_s)

        # ---- R^T = V^T w  [d, tok] -----------------------------------------
        rsb = []
        for m in range(NDT):
            ps = ps_acc.tile([P, TB], F32, tag="acc", name="ps")
            for s in range(NST):
                nc.tensor.matmul(
                    ps, vsb[s][:, m * P:(m + 1) * P], wsb[s],
                    start=(s == 0), stop=(s == NST - 1),
                )
            r_m = rpool.tile([P, TB], F32, tag="rsb", name="r_m")
            nc.scalar.activation(out=r_m, in_=ps, func=Act.Copy)
            rsb.append(r_m)

        # ---- gate + blend ---------------------------------------------------
        for m in range(NDT):
            ps = ps_acc.tile([P, TB], F32, tag="acc", name="ps")
            for k in range(NDT):
                nc.tensor.matmul(
                    ps, wg_sb[k][:, m * P:(m + 1) * P], xts[k],
                    start=(k == 0), stop=False,
                )
            for s in range(NST):
                nc.tensor.matmul(
                    ps, w2vt[s][:, m * P:(m + 1) * P], wsb[s],
                    start=False, stop=(s == NST - 1),
                )
            g_m = gpool.tile([P, TB], F32, tag="gsb", name="g_m")
            nc.scalar.activation(
                out=g_m, in_=ps, func=Act.Sigmoid, bias=bg_sb[:, m:m + 1],
            )
            # O = R + g * (X - R)
            d_m = tpool.tile([P, TB], F32, tag="tt", name="d_m")
            nc.vector.tensor_sub(out=d_m, in0=_f32(xts[m]), in1=rsb[m])
            nc.vector.tensor_mul(out=d_m, in0=d_m, in1=g_m)
            o_m = opool.tile([P, TB], F32, tag="osb", name="o_m")
            nc.vector.tensor_add(out=o_m, in0=d_m, in1=rsb[m])
            nc.sync.dma_start(out=ot[m * P:(m + 1) * P, col], in_=o_m)


def kernel(mamba_states, memory, Wq, bq, Wk, bk, Wv, bv, Wg, bg):
    global LAST_RESULTS
    if "nc" not in _CACHE:
        _CACHE["nc"] = _build()
    nc = _CACHE["nc"]

    f = np.ascontiguousarray
    shared = {
        "memt": f(np.asarray(memory, np.float32).T),
        "wq": f(np.asarray(Wq, np.float32)),
        "wk": f(np.asarray(Wk, np.float32)),
        "wv": f(np.asarray(Wv, np.float32)),
        "wg": f(np.asarray(Wg, np.float32)),
        "bq": f(np.asarray(bq, np.float32)),
        "bk": f(np.asarray(bk, np.float32)),
        "bv": f(np.asarray(bv, np.float32)),
        "bg": f(np.asarray(bg, np.float32)),
    }
    states = np.asarray(mamba_states, np.float32)
    in_maps = [dict(shared, xt=f(states[i].T)) for i in range(B)]

    res = run_bass_kernel_spmd(
        nc, in_maps, list(range(B)), trace=_CACHE.get("trace", False)
    )
    LAST_RESULTS = res
    out = np.stack([res.results[i]["ot"].T for i in range(B)])
    return np.ascontiguousarray(out)


# revision 28
# speedup vs baseline: 11.0461x; 11.0461x over previous
"""Differentiable episodic memory retrieval kernel for Trainium2 (8 NeuronCores).

Shards mamba_states over batch (1 batch element per core); memory matrix and
projection weights are replicated. All device tensors use a feature-major
("transposed") layout [d, tokens] so every matmul contracts over the SBUF
partition dimension.

Math (per core, X = states^T [d, tok]):
  Q^T = Wq^T X + bq                  (f32r matmuls)
  c_t = 1/||Q_t||                    (Square + ones-matmul partition reduction)
  S^T[n,t] = K^T(d,n) . Q^T(d,t)     (bf16; K unnormalized)
  E = exp(S * c_t * kscale_n)        (kscale_n = 1/(sqrt(d)*||K_n||), ACT scale)
  w = E / sum_n E                    (ones-matmul sums, K=1 matmul broadcast)
  R^T = V^T w  (V includes bv; softmax weights sum to 1 so bias passes through)
  G = Wg1^T X + W2V^T w + bg         (W2V^T = V Wg2 precomputed in preamble)
  O = R + sigmoid(G) * (X - R)
"""

import numpy as np

import concourse.bass as bass
import concourse.mybir as mybir
import concourse.tile as tile
from concourse import bacc
from concourse.bass_utils import run_bass_kernel_spmd

B, T, D = 8, 4096, 1024
NS = 512          # memory slots
TB = 512          # tokens per block
NBLK = T // TB    # 8
NDT = D // 128    # 8 tiles along d
NST = NS // 128   # 4 tiles along slots
P = 128
H = D // 2

F32 = mybir.dt.float32
F32R = mybir.dt.float32r
BF16 = mybir.dt.bfloat16

_CACHE = {}
LAST_RESULTS = None


def _f32(ap):
    return ap.bitcast(F32)


def _build():
    from contextlib import ExitStack

    nc = bacc.Bacc("TRN2", target_bir_lowering=False, debug=False)

    # f32r dram tensors: fed straight into f32r matmuls (same bits as f32)
    xt = nc.dram_tensor("xt", [D, T], F32R, kind="ExternalInput").ap()
    memt = nc.dram_tensor("memt", [D, NS], F32R, kind="ExternalInput").ap()
    wq = nc.dram_tensor("wq", [D, D], F32R, kind="ExternalInput").ap()
    wk = nc.dram_tensor("wk", [D, D], F32R, kind="ExternalInput").ap()
    wv = nc.dram_tensor("wv", [D, D], F32R, kind="ExternalInput").ap()
    wg = nc.dram_tensor("wg", [2 * D, D], F32R, kind="ExternalInput").ap()
    bq = nc.dram_tensor("bq", [D], F32, kind="ExternalInput").ap()
    bk = nc.dram_tensor("bk", [D], F32, kind="ExternalInput").ap()
    bv = nc.dram_tensor("bv", [D], F32, kind="ExternalInput").ap()
    bg = nc.dram_tensor("bg", [D], F32, kind="ExternalInput").ap()
    ot = nc.dram_tensor("ot", [D, T], F32, kind="ExternalOutput").ap()

    with tile.TileContext(nc) as tc, ExitStack() as ctx:
        _body(nc, tc, ctx, xt, memt, wq, wk, wv, wg, bq, bk, bv, bg, ot)

    nc.compile()
    return nc


def _body(nc, tc, ctx, xt, memt, wq, wk, wv, wg, bq, bk, bv, bg, ot):
    Act = mybir.ActivationFunctionType

    singles = ctx.enter_context(tc.tile_pool(name="singles", bufs=1))
    wpool = ctx.enter_context(tc.tile_pool(name="weights", bufs=1))
    xpool = ctx.enter_context(tc.tile_pool(name="xt", bufs=12))

    # --- preamble-critical loads first: K projection gates the pipeline -----
    pre_cm = tc.tile_pool(name="pre", bufs=1)
    pre = pre_cm.__enter__()
    mem_sb = [pre.tile([P, NS], F32R, tag=f"mem{i}", name="mem_sb")
              for i in range(NDT)]
    for k in range(NDT):
        nc.sync.dma_start(out=mem_sb[k], in_=memt[k * P:(k + 1) * P, :])
    wk_sb = [pre.tile([P, D], F32R, tag=f"wk{i}", name="wk_sb")
             for i in range(NDT)]
    for k in range(NDT):
        nc.sync.dma_start(out=wk_sb[k], in_=wk[k * P:(k + 1) * P, :])

    # block-0 activations early so Q can fill preamble gaps
    xts0 = [xpool.tile([P, TB], F32R, tag="xt", name="xts") for _ in range(NDT)]
    for k in range(NDT):
        nc.sync.dma_start(out=xts0[k], in_=xt[k * P:(k + 1) * P, 0:TB])

    # --- constants -----------------------------------------------------------
    ones_col = singles.tile([P, 1], BF16)          # lhsT for partition sums
    nc.vector.memset(ones_col, 1.0)
    ones_row = singles.tile([1, P], F32)           # lhsT for partition bcast
    nc.vector.memset(ones_row, 1.0)
    ones_row_bf = singles.tile([1, P], BF16)
    nc.vector.memset(ones_row_bf, 1.0)

    # per-partition bias tiles: [p, t] = b[t*128 + p]
    bq_sb = singles.tile([P, NDT], F32)
    nc.sync.dma_start(out=bq_sb, in_=bq.rearrange("(t p) -> p t", p=P))
    bk_sb = singles.tile([P, NDT], F32)
    nc.sync.dma_start(out=bk_sb, in_=bk.rearrange("(t p) -> p t", p=P))
    bg_sb = singles.tile([P, NDT], F32)
    nc.sync.dma_start(out=bg_sb, in_=bg.rearrange("(t p) -> p t", p=P))
    # bv broadcast across partitions: [128, D]
    bv_bc = singles.tile([P, D], F32)
    nc.sync.dma_start(
        out=bv_bc,
        in_=bass.AP(tensor=bv.tensor, offset=bv.offset, ap=[[0, P], [1, D]]),
    )

    # --- resident weights ----------------------------------------------------
    wq_sb = [wpool.tile([P, D], F32R, tag=f"wq{i}", name="wq_sb") for i in range(NDT)]
    for k in range(NDT):
        nc.sync.dma_start(out=wq_sb[k], in_=wq[k * P:(k + 1) * P, :])
    # wv/wg2 queue before wg: they reuse wk slots and are needed sooner
    wv_sb = [pre.tile([P, D], F32R, tag=f"wk{i}", name="wv_sb")
             for i in range(NDT)]
    for k in range(NDT):
        nc.sync.dma_start(out=wv_sb[k], in_=wv[k * P:(k + 1) * P, :])
    wg2_sb = [pre.tile([P, D], F32R, tag=f"wk{i}", name="wg2_sb")
              for i in range(NDT)]
    for k in range(NDT):
        nc.sync.dma_start(out=wg2_sb[k], in_=wg[D + k * P:D + (k + 1) * P, :])

    wg_sb = [wpool.tile([P, D], F32R, tag=f"wg{i}", name="wg_sb") for i in range(NDT)]
    for k in range(NDT):
        nc.sync.dma_start(out=wg_sb[k], in_=wg[k * P:(k + 1) * P, :])

    # static attention operands produced by the preamble
    ksb = [wpool.tile([P, NS], BF16, tag=f"ksb{i}", name="ksb") for i in range(NDT)]
    vsb = [wpool.tile([P, D], BF16, tag=f"vsb{i}", name="vsb") for i in range(NST)]
    w2vt = [wpool.tile([P, D], BF16, tag=f"w2vt{i}", name="w2vt") for i in range(NST)]
    kscale = [wpool.tile([P, 1], F32, tag=f"ksc{i}", name="kscale") for i in range(NST)]

    # =========================================================================
    # Preamble: K / V projections of the memory matrix, W2V^T = V @ Wg2
    # =========================================================================
    with tc.tile_pool(name="pre_ps", bufs=2, space="PSUM") as pre_ps, \
         tc.tile_pool(name="pre_tmp", bufs=2) as pre_tmp:
        # K^T feature-major [d, slots], bias added, cast to bf16
        for m in range(NDT):
            ps = pre_ps.tile([P, NS], F32, tag="pps", name="ps")
            for k in range(NDT):
                nc.tensor.matmul(
                    ps, wk_sb[k][:, m * P:(m + 1) * P], mem_sb[k],
                    start=(k == 0), stop=(k == NDT - 1),
                )
            nc.scalar.activation(
                out=ksb[m], in_=ps, func=Act.Identity, bias=bk_sb[:, m:m + 1],
            )

        # per-slot 1/(sqrt(d)*||K_n||) from feature-major K^T:
        # Square(ksb) -> ones-matmul over d -> [1, slots] -> PE transpose
        kss_ps = pre_ps.tile([1, NS], F32, tag="kssp", name="kss_ps")
        for m in range(NDT):
            ksq = pre_tmp.tile([P, NS], BF16, tag="ksq")
            nc.scalar.activation(out=ksq, in_=ksb[m], func=Act.Square)
            nc.tensor.matmul(kss_ps, ones_col, ksq,
                             start=(m == 0), stop=(m == NDT - 1))
        # ln/exp rsqrt on the [1, slots] row, then transpose to [128, NST]
        kroot = pre_tmp.tile([1, NS], F32, tag="kroot")
        nc.scalar.activation(out=kroot, in_=kss_ps, func=Act.Ln, scale=float(D))
        kscale_row = pre_tmp.tile([1, NS], F32, tag="kscrow")
        nc.scalar.activation(out=kscale_row, in_=kroot, func=Act.Exp, scale=-0.5)
        ident1 = pre_tmp.tile([1, 1], F32, tag="id1")
        nc.vector.memset(ident1, 1.0)
        for s in range(NST):
            kt_ps = pre_ps.tile([P, 1], F32, tag="ktp", name="kt_ps")
            nc.tensor.transpose(
                kt_ps, kscale_row[0:1, s * P:(s + 1) * P], ident1,
            )
            nc.vector.tensor_copy(out=kscale[s], in_=kt_ps)

        # V slot-major [slots, d], bias added directly (softmax weights sum to
        # one, so R = w @ (V0 + bv) = w @ V0 + bv matches the reference)
        for s in range(NST):
            vtmp = pre_tmp.tile([P, D], F32, tag="vtmp")
            for h in range(2):
                ps = pre_ps.tile([P, H], F32, tag="pps", name="ps")
                for k in range(NDT):
                    nc.tensor.matmul(
                        ps,
                        mem_sb[k][:, s * P:(s + 1) * P],
                        wv_sb[k][:, h * H:(h + 1) * H],
                        start=(k == 0), stop=(k == NDT - 1),
                    )
                nc.vector.tensor_add(
                    out=vtmp[:, h * H:(h + 1) * H], in0=ps,
                    in1=bv_bc[:, h * H:(h + 1) * H],
                )
            nc.vector.tensor_copy(out=vsb[s], in_=vtmp)

        # V^T feature-major (bf16, transient) by PE-transposing V slot-major
        identp = pre_tmp.tile([P, P], BF16, tag="idp")
        from concourse.masks import make_identity
        make_identity(nc, identp)
        vt_bf = [pre_tmp.tile([P, NS], BF16, tag=f"vt{i}", bufs=1, name="vt_bf")
                 for i in range(NDT)]
        for m in range(NDT):
            for s in range(NST):
                tp = pre_ps.tile([P, P], BF16, tag="ktp", name="tp")
                nc.tensor.transpose(
                    tp, vsb[s][:, m * P:(m + 1) * P], identp,
                )
                nc.vector.tensor_copy(
                    out=vt_bf[m][:, s * P:(s + 1) * P], in_=tp,
                )

        # wg2 in bf16 (transient): ACT copy from f32r load
        wg2_bf = [pre_tmp.tile([P, D], BF16, tag=f"wg2b{i}", bufs=1, name="wg2_bf")
                  for i in range(NDT)]
        for k in range(NDT):
            nc.scalar.activation(out=wg2_bf[k], in_=_f32(wg2_sb[k]), func=Act.Copy)

        # W2V^T slot-major [slots, dout] = V @ Wg2   (bf16)
        for s in range(NST):
            for h in range(2):
                ps = pre_ps.tile([P, H], F32, tag="pps", name="ps")
                for k in range(NDT):
                    nc.tensor.matmul(
                        ps, vt_bf[k][:, s * P:(s + 1) * P],
                        wg2_bf[k][:, h * H:(h + 1) * H],
                        start=(k == 0), stop=(k == NDT - 1),
                    )
                nc.scalar.activation(
                    out=w2vt[s][:, h * H:(h + 1) * H], in_=ps, func=Act.Copy,
                )

    pre_cm.__exit__(None, None, None)

    # =========================================================================
    # Main loop over token blocks
    # =========================================================================
    qpool = ctx.enter_context(tc.tile_pool(name="q", bufs=10))
    qqpool = ctx.enter_context(tc.tile_pool(name="qq", bufs=3))
    epool = ctx.enter_context(tc.tile_pool(name="e", bufs=5))
    gpool = ctx.enter_context(tc.tile_pool(name="g", bufs=4))
    tpool = ctx.enter_context(tc.tile_pool(name="tmp", bufs=4))
    opool = ctx.enter_context(tc.tile_pool(name="o", bufs=4))
    bpool = ctx.enter_context(tc.tile_pool(name="bcast", bufs=1))
    spool = ctx.enter_context(tc.tile_pool(name="small", bufs=2))

    ps_acc = ctx.enter_context(tc.tile_pool(name="ps_acc", bufs=2, space="PSUM"))
    ps_g = ctx.enter_context(tc.tile_pool(name="ps_g", bufs=3, space="PSUM"))
    ps_s = ctx.enter_context(tc.tile_pool(name="ps_s", bufs=2, space="PSUM"))
    ps_sm = ctx.enter_context(tc.tile_pool(name="ps_sm", bufs=1, space="PSUM"))

    for blk in range(NBLK):
        col = slice(blk * TB, (blk + 1) * TB)

        if blk == 0:
            xts = xts0
        else:
            xts = [xpool.tile([P, TB], F32R, tag="xt", name="xts")
                   for _ in range(NDT)]
            for k in range(NDT):
                nc.sync.dma_start(out=xts[k], in_=xt[k * P:(k + 1) * P, col])

        # ---- Q projection + sum of squares ---------------------------------
        qss_ps = ps_sm.tile([1, TB], F32, tag="sm", name="qss_ps")
        qsb = []
        for m in range(NDT):
            ps = ps_acc.tile([P, TB], F32, tag="acc", name="ps")
            for k in range(NDT):
                nc.tensor.matmul(
                    ps, wq_sb[k][:, m * P:(m + 1) * P], xts[k],
                    start=(k == 0), stop=(k == NDT - 1),
                )
            q_m = qpool.tile([P, TB], BF16, tag="qsb", name="q_m")
            nc.scalar.activation(
                out=q_m, in_=ps, func=Act.Identity, bias=bq_sb[:, m:m + 1],
            )
            qq_m = qqpool.tile([P, TB], BF16, tag="qsq", name="qq_m")
            nc.vector.tensor_mul(out=qq_m, in0=q_m, in1=q_m)
            nc.tensor.matmul(
                qss_ps, ones_col, qq_m,
                start=(m == 0), stop=(m == NDT - 1),
            )
            qsb.append(q_m)

        # ---- per-token scale c = 1/||Q_t|| = exp(-0.5 ln(qss)) -------------
        # (ln+exp live in one ACT table set; avoids sqrt-set swaps)
        ln_q = spool.tile([1, TB], F32, tag="lnq")
        nc.scalar.activation(out=ln_q, in_=qss_ps, func=Act.Ln)
        c_sb = spool.tile([1, TB], BF16, tag="c")
        nc.scalar.activation(out=c_sb, in_=ln_q, func=Act.Exp, scale=-0.5)
        cb_ps = ps_sm.tile([P, TB], F32, tag="sm", name="cb_ps")
        nc.tensor.matmul(cb_ps, ones_row_bf, c_sb, start=True, stop=True)
        cb_sb = bpool.tile([P, TB], F32, tag="cb_sb")
        nc.scalar.activation(out=cb_sb, in_=cb_ps, func=Act.Copy)

        # ---- scores^T [slots, tok] -> exp (s-outer: 2 psum banks) ----------
        ss_ps = ps_sm.tile([1, TB], F32, tag="sm", name="ss_ps")
        esb = []
        for s in range(NST):
            s_ps = ps_s.tile([P, TB], F32, tag="pss", name="s_ps")
            for m in range(NDT):
                nc.tensor.matmul(
                    s_ps, ksb[m][:, s * P:(s + 1) * P], qsb[m],
                    start=(m == 0), stop=(m == NDT - 1),
                )
            e_in = tpool.tile([P, TB], F32, tag="tt", name="e_in")
            nc.vector.tensor_mul(out=e_in, in0=s_ps, in1=cb_sb)
            e_s = epool.tile([P, TB], BF16, tag="esb", name="e_s")
            nc.scalar.activation(out=e_s, in_=e_in, func=Act.Exp, scale=kscale[s])
            nc.tensor.matmul(
                ss_ps, ones_col, e_s, start=(s == 0), stop=(s == NST - 1),
            )
            esb.append(e_s)

        # ---- gate X-half: no softmax dependency, fills the softmax gap -----
        g_ps = []
        for m in range(NDT):
            ps = ps_g.tile([P, TB], F32, tag="psg", name="ps")
            for k in range(NDT):
                nc.tensor.matmul(
                    ps, wg_sb[k][:, m * P:(m + 1) * P], xts[k],
                    start=(k == 0), stop=False,
                )
            g_ps.append(ps)

        # ---- normalized weights w = E / sum --------------------------------
        rs_sb = spool.tile([1, TB], F32, tag="rs")
        rs_scr = spool.tile([1, TB], F32, tag="rs_scr")
        nc.vector.reciprocal_approx_accurate(out=rs_sb, in_=ss_ps, scratch=rs_scr)
        rb_ps = ps_sm.tile([P, TB], F32, tag="sm", name="rb_ps")
        nc.tensor.matmul(rb_ps, ones_row, rs_sb, start=True, stop=True)
        wsb = []
        for s in range(NST):
            w_s = epool.tile([P, TB], BF16, tag="wsb", name="w_s")
            nc.vector.tensor_mul(out=w_s, in0=rb_ps, in1=esb[s])
            wsb.append(w_s)

        # ---- R^T = V^T w  [d, tok] (kept in PSUM, read by the blend) -------
        rsb = []
        for m in range(NDT):
            ps = ps_acc.tile([P, TB], F32, tag="acc", name="ps")
            for s in range(NST):
                nc.tensor.matmul(
                    ps, vsb[s][:, m * P:(m + 1) * P], wsb[s],
                    start=(s == 0), stop=(s == NST - 1),
                )
            rsb.append(ps)

        # ---- gate completion (W2V^T w) + sigmoid + blend --------------------
        for m in range(NDT):
            ps = g_ps[m]
            for s in range(NST):
                nc.tensor.matmul(
                    ps, w2vt[s][:, m * P:(m + 1) * P], wsb[s],
                    start=False, stop=(s == NST - 1),
                )
            g_m = gpool.tile([P, TB], F32, tag="gsb", name="g_m")
            nc.scalar.activation(
                out=g_m, in_=ps, func=Act.Sigmoid, bias=bg_sb[:, m:m + 1],
            )
            # O = R + g * (X - R)
            d_m = tpool.tile([P, TB], F32, tag="tt", name="d_m")
            nc.vector.tensor_sub(out=d_m, in0=_f32(xts[m]), in1=rsb[m])
            nc.vector.tensor_mul(out=d_m, in0=d_m, in1=g_m)
            o_m = opool.tile([P, TB], F32, tag="osb", name="o_m")
            nc.vector.tensor_add(out=o_m, in0=d_m, in1=rsb[m])
            nc.sync.dma_start(out=ot[m * P:(m + 1) * P, col], in_=o_m)


def kernel(mamba_states, memory, Wq, bq, Wk, bk, Wv, bv, Wg, bg):
    global LAST_RESULTS
    if "nc" not in _CACHE:
        _CACHE["nc"] = _build()
    nc = _CACHE["nc"]

    f = np.ascontiguousarray
    shared = {
        "memt": f(np.asarray(memory, np.float32).T),
        "wq": f(np.asarray(Wq, np.float32)),
        "wk": f(np.asarray(Wk, np.float32)),
        "wv": f(np.asarray(Wv, np.float32)),
        "wg": f(np.asarray(Wg, np.float32)),
        "bq": f(np.asarray(bq, np.float32)),
        "bk": f(np.asarray(bk, np.float32)),
        "bv": f(np.asarray(bv, np.float32)),
        "bg": f(np.asarray(bg, np.float32)),
    }
    states = np.asarray(mamba_states, np.float32)
    in_maps = [dict(shared, xt=f(states[i].T)) for i in range(B)]

    res = run_bass_kernel_spmd(
        nc, in_maps, list(range(B)), trace=_CACHE.get("trace", False)
    )
    LAST_RESULTS = res
    out = np.stack([res.results[i]["ot"].T for i in range(B)])
    return np.ascontiguousarray(out)
